# revision 5
# baseline (speedup 1.0000x reference)
"""Trainium2 Bass kernel for the ConsistencyUNet MLP (B=131072 data-parallel over 8 cores).

Layout: activations live as [C=128 partitions, N columns] f32r in SBUF.
GroupNorm(G=8, 16 ch/group) is computed as:
  x_c   = P @ x            (P = I - group-mean matrix; often folded into conv weights)
  var_g = Gv @ (x_c + b)^2 (Gv = group indicator / 16, M=8 stats matmul)
  inv   = 1/sqrt(var+eps)  (ACT Sqrt + DVE reciprocal on [8,N])
  invb  = B8 @ inv         (K=8 broadcast matmul)
  a     = SiLU((x_c + b) * invb)
All per-channel biases ride as host-folded constants applied via ACT bias /
scalar_tensor_tensor per-partition scalars; they are never materialized on device.
"""

import numpy as np
from contextlib import ExitStack

import concourse.bass as bass
import concourse.bacc as bacc
import concourse.tile as tile
import concourse.mybir as mybir
from concourse.bass_utils import run_bass_kernel_spmd

FP32 = mybir.dt.float32
F32R = mybir.dt.float32r
AF = mybir.ActivationFunctionType
ALU = mybir.AluOpType

N_CORES = 8
B = 131072
BC = B // N_CORES          # 16384 columns per core
FT = 512                   # columns per tile
NT = BC // FT              # 32 tiles per core
G, H, SE, OBS, ACT_D = 8, 128, 64, 60, 6
CIN = OBS + ACT_D          # 66
EPS = 1e-5




# ---------------------------------------------------------------------------
# Host-side weight folding.
def _fold_params(params):
    """Returns dict of fp32 arrays ready for the device program."""
    f64 = lambda a: np.asarray(a, dtype=np.float64)
    P = np.eye(H) - np.kron(np.eye(G), np.full((H // G, H // G), 1.0 / (H // G)))

    out = {}
    half = SE // 2
    freqs = np.exp(-np.log(10000.0) * np.arange(half) / (half - 1))
    out["freqs_dup"] = np.concatenate([freqs, freqs]).astype(np.float32)[:, None]
    out["phase"] = np.concatenate([np.zeros(half), np.full(half, np.pi / 2)]).astype(
        np.float32
    )[:, None]

    s1w, s1b = f64(params["s1w"]), f64(params["s1b"])
    s2w, s2b = f64(params["s2w"]), f64(params["s2b"])
    out["s1wT"] = s1w.T.astype(np.float32)          # [SE, 2SE] lhsT (K=64, M=128)
    out["s1b"] = s1b.astype(np.float32)[:, None]

    out["inwT"] = f64(params["inw"]).T.astype(np.float32)   # [66, 128]
    cum = f64(params["inb"]).copy()                          # running channel bias of x_mat

    blocks = []
    for name in ["down1", "down2", "down3", "mid", "up3", "up2", "up1"]:
        p = params[name]
        C1 = f64(p["c1w"])[:, :, 1]
        C2 = f64(p["c2w"])[:, :, 1]
        spw, spb = f64(p["spw"]), f64(p["spb"])
        c1b, c2b = f64(p["c1b"]), f64(p["c2b"])
        M = spw @ s2w
        d = c1b + spw @ s2b + spb
        blocks.append(
            {
                "name": name,
                "PC1T": (P @ C1).T.astype(np.float32),   # lhsT [K=128, M=128]
                "PMT": (P @ M).T.astype(np.float32),
                "pd": (P @ d).astype(np.float32)[:, None],
                "C2T": C2.T.astype(np.float32),
                "c2b": c2b,
            }
        )

    ups = {}
    for name in ["up3", "up2", "up1"]:
        U = f64(params[name + "w"])[:, :, 0]          # [128, 256]
        ups[name] = {
            "UaT": U[:, :H].T.astype(np.float32),      # m-part lhsT [128,128]
            "UbT": U[:, H:].T.astype(np.float32),      # h-part lhsT
            "U": U,
            "ub": f64(params[name + "b"]),
        }

    # Walk the net to compute the running bias (cum) entering each GN.
    pcum = []  # per down/mid/up block: bias of x_mat entering gn1 (P-projected and raw)
    for i, bl in enumerate(blocks):
        pcum.append(
            {"cum": cum.copy(), "pcum": (P @ cum).astype(np.float32)[:, None]}
        )
        cum = cum + bl["c2b"]
        if bl["name"] == "mid":
            cum_m = cum.copy()
        if bl["name"] == "down3":
            cum_h3 = cum.copy()
        if bl["name"] == "down2":
            cum_h2 = cum.copy()
        if bl["name"] == "down1":
            cum_h1 = cum.copy()
        if bl["name"] == "up3":
            cum_u3 = cum.copy()
        if bl["name"] == "up2":
            cum_u2 = cum.copy()
        # at up-block entries, cum resets to the up-projection image
        if bl["name"] == "mid":
            u = ups["up3"]
            cum = u["U"] @ np.concatenate([cum_m, cum_h3]) + u["ub"]
        if bl["name"] == "up3":
            u = ups["up2"]
            cum = u["U"] @ np.concatenate([cum_u3, cum_h2]) + u["ub"]
        if bl["name"] == "up2":
            u = ups["up1"]
            cum = u["U"] @ np.concatenate([cum_u2, cum_h1]) + u["ub"]

    final_cum = cum  # bias of u1_mat
    out["final_pcum"] = (P @ final_cum).astype(np.float32)[:, None]
    out["blocks"] = blocks
    out["pcum"] = pcum
    out["ups"] = ups
    out["PT"] = P.T.astype(np.float32)               # lhsT for x_c = P @ x
    out["outwT"] = f64(params["outw"]).T.astype(np.float32)  # [128, 6]
    out["outb"] = np.asarray(params["outb"], dtype=np.float32)

    Gv = np.zeros((H, G), dtype=np.float32)
    for g in range(G):
        Gv[g * 16 : (g + 1) * 16, g] = 1.0 / 16.0
    out["Gv"] = Gv                                    # lhsT [K=128, M=8]
    out["B8"] = np.kron(np.eye(G), np.ones((1, 16))).astype(np.float32)  # [8,128]
    out["ones64"] = np.ones((1, SE), dtype=np.float32)
    return out


# ---------------------------------------------------------------------------
# Device program.
def _build_program(n_tiles=NT):
    nc = bacc.Bacc("TRN2", target_bir_lowering=False, debug=False)
    FC = n_tiles * FT

    inp_d = nc.dram_tensor("inp", [CIN, FC], FP32, kind="ExternalInput").ap()
    sig_d = nc.dram_tensor("sig", [1, FC], FP32, kind="ExternalInput").ap()
    # stacked matmul weights (lhsT layout), biases, constants
    wm_names = []
    out_d = nc.dram_tensor("out", [ACT_D, FC], FP32, kind="ExternalOutput").ap()

    # weight DRAM tensors
    w_d = {}

    def wdram(name, shape):
        w_d[name] = nc.dram_tensor("w_" + name, list(shape), FP32, kind="ExternalInput").ap()
        return w_d[name]

    wdram("inwT", [CIN, H])
    wdram("s1wT", [SE, 2 * SE])
    wdram("PT", [H, H])
    wdram("Gv", [H, G])
    wdram("B8", [G, H])
    wdram("ones64", [1, SE])
    wdram("outwT", [H, ACT_D])
    wdram("freqs_dup", [SE, 1])
    wdram("phase", [SE, 1])
    wdram("s1b", [2 * SE, 1])
    wdram("final_pcum", [H, 1])
    wdram("epsv", [G, 1])
    for i in range(7):
        wdram(f"PC1T_{i}", [H, H])
        wdram(f"PMT_{i}", [H, H])
        wdram(f"C2T_{i}", [H, H])
        wdram(f"pd_{i}", [H, 1])
        wdram(f"pcum_{i}", [H, 1])
    for name in ["up3", "up2", "up1"]:
        wdram(f"UaT_{name}", [H, H])
        wdram(f"UbT_{name}", [H, H])

    with tile.TileContext(nc) as tc:
        with ExitStack() as ctx:
            wpool = ctx.enter_context(tc.tile_pool(name="w", bufs=1))
            # Load all weights once, rounding matmul operands to f32r.
            w = {}
            for name, d in w_d.items():
                shp = list(d.shape)
                raw = wpool.tile(shp, FP32, tag="wraw_" + name)
                nc.sync.dma_start(raw[:], d[:])
                if shp[1] == 1:
                    w[name] = raw  # bias vectors stay fp32
                else:
                    r = wpool.tile(shp, F32R, tag="w_" + name)
                    nc.vector.tensor_copy(r[:], raw[:])
                    w[name] = r

            io = ctx.enter_context(tc.tile_pool(name="io", bufs=3))
            act = ctx.enter_context(tc.tile_pool(name="act", bufs=2))
            st = ctx.enter_context(tc.tile_pool(name="st", bufs=2))
            ps = ctx.enter_context(tc.tile_pool(name="ps", bufs=2, space="PSUM"))

            def groupnorm_silu(xc_psum, pcum_ap, tag):
                """xc_psum: [128,FT] PSUM holding centered pre-bias x.
                pcum_ap: [128,1] fp32 bias AP or None. Returns SBUF f32r tile a = SiLU(gn)."""
                sq = act.tile([H, FT], F32R, tag=f"sq")
                if pcum_ap is not None:
                    nc.scalar.activation(sq[:], xc_psum[:], AF.Square, bias=pcum_ap)
                else:
                    nc.scalar.activation(sq[:], xc_psum[:], AF.Square)
                s2 = ps.tile([G, FT], FP32, tag="S")
                nc.tensor.matmul(s2[:], w["Gv"][:], sq[:], start=True, stop=True)
                sd = st.tile([G, FT], FP32, tag="sd")
                nc.scalar.activation(sd[:], s2[:], AF.Sqrt, bias=w["epsv"][:])
                inv = st.tile([G, FT], F32R, tag="inv")
                with nc.allow_low_precision(reason="f32r rounding for matmul rhs"):
                    nc.vector.reciprocal(inv[:], sd[:])
                invb = ps.tile([H, FT], FP32, tag="I")
                nc.tensor.matmul(invb[:], w["B8"][:], inv[:], start=True, stop=True)
                invc = act.tile([H, FT], FP32, tag="invc")
                nc.scalar.activation(invc[:], invb[:], AF.Copy)
                z = act.tile([H, FT], FP32, tag="z")
                if pcum_ap is not None:
                    nc.vector.scalar_tensor_tensor(
                        z[:], xc_psum[:], pcum_ap, invc[:], ALU.add, ALU.mult
                    )
                else:
                    nc.vector.tensor_mul(z[:], xc_psum[:], invc[:])
                a = act.tile([H, FT], F32R, tag="a")
                nc.scalar.activation(a[:], z[:], AF.Silu)
                return a

            for it in range(n_tiles):
                cs = bass.ts(it, FT)

                # ---- input projection ----
                inp_raw = io.tile([CIN, FT], FP32, tag="inp_raw")
                nc.sync.dma_start(inp_raw[:], inp_d[:, cs])
                inp_r = io.tile([CIN, FT], F32R, tag="inp_r")
                nc.scalar.activation(inp_r[:], inp_raw[:], AF.Copy)
                x_ps = ps.tile([H, FT], FP32, tag="B")
                nc.tensor.matmul(x_ps[:], w["inwT"][:], inp_r[:], start=True, stop=True)
                x = act.tile([H, FT], F32R, tag="x0")
                nc.scalar.activation(x[:], x_ps[:], AF.Copy)

                # ---- sigma embedding path ----
                sig_raw = io.tile([1, FT], FP32, tag="sig_raw")
                nc.sync.dma_start(sig_raw[:], sig_d[:, cs])
                sig_r = io.tile([1, FT], F32R, tag="sig_r")
                nc.scalar.activation(sig_r[:], sig_raw[:], AF.Copy)
                sig_b = ps.tile([SE, FT], FP32, tag="B")
                nc.tensor.matmul(sig_b[:], w["ones64"][:], sig_r[:], start=True, stop=True)
                emb = act.tile([SE, FT], F32R, tag="emb")
                nc.scalar.activation(
                    emb[:], sig_b[:], AF.Sin,
                    scale=w["freqs_dup"][:], bias=w["phase"][:],
                )
                zse = ps.tile([H, FT], FP32, tag="B")
                nc.tensor.matmul(zse[:], w["s1wT"][:], emb[:], start=True, stop=True)
                sz = act.tile([H, FT], F32R, tag="sz")
                nc.scalar.activation(sz[:], zse[:], AF.Silu, bias=w["s1b"][:])

                # ---- residual blocks ----
                saved = {}

                def res_block(i, x, otag):
                    # gn1: xc = P @ x
                    xc = ps.tile([H, FT], FP32, tag="A")
                    nc.tensor.matmul(xc[:], w["PT"][:], x[:], start=True, stop=True)
                    a1 = groupnorm_silu(xc, w[f"pcum_{i}"][:], f"b{i}g1")
                    # conv1 (centered) + sp accumulate
                    hc = ps.tile([H, FT], FP32, tag="A")
                    nc.tensor.matmul(hc[:], w[f"PC1T_{i}"][:], a1[:], start=True, stop=False)
                    nc.tensor.matmul(hc[:], w[f"PMT_{i}"][:], sz[:], start=False, stop=True)
                    a2 = groupnorm_silu(hc, w[f"pd_{i}"][:], f"b{i}g2")
                    # conv2 + residual
                    y2 = ps.tile([H, FT], FP32, tag="B")
                    nc.tensor.matmul(y2[:], w[f"C2T_{i}"][:], a2[:], start=True, stop=True)
                    xn = act.tile([H, FT], F32R, tag=otag)
                    nc.vector.tensor_add(xn[:], x[:], y2[:])
                    return xn

                h1 = res_block(0, x, "h1")
                saved["h1"] = h1
                h2 = res_block(1, h1, "h2")
                saved["h2"] = h2
                h3 = res_block(2, h2, "h3")
                saved["h3"] = h3
                m = res_block(3, h3, "m")

                def up_block(i, name, a_t, b_t, otag):
                    up_ps = ps.tile([H, FT], FP32, tag="B")
                    nc.tensor.matmul(up_ps[:], w[f"UaT_{name}"][:], a_t[:], start=True, stop=False)
                    nc.tensor.matmul(up_ps[:], w[f"UbT_{name}"][:], b_t[:], start=False, stop=True)
                    ux = act.tile([H, FT], F32R, tag="ux")
                    nc.scalar.activation(ux[:], up_ps[:], AF.Copy)
                    return res_block(i, ux, otag)

                u3 = up_block(4, "up3", m, saved["h3"], "u3")
                u2 = up_block(5, "up2", u3, saved["h2"], "u2")
                u1 = up_block(6, "up1", u2, saved["h1"], "u1")

                # ---- final GN + out projection ----
                xcf = ps.tile([H, FT], FP32, tag="A")
                nc.tensor.matmul(xcf[:], w["PT"][:], u1[:], start=True, stop=True)
                af = groupnorm_silu(xcf, w["final_pcum"][:], "fin")
                out_ps = ps.tile([ACT_D, FT], FP32, tag="B")
                nc.tensor.matmul(out_ps[:], w["outwT"][:], af[:], start=True, stop=True)
                out_sb = io.tile([ACT_D, FT], FP32, tag="out_sb")
                nc.scalar.activation(out_sb[:], out_ps[:], AF.Copy)
                nc.sync.dma_start(out_d[:, cs], out_sb[:])

    nc.compile()
    return nc


# ---------------------------------------------------------------------------
def _flat_weight_maps(fold):
    m = {
        "w_inwT": fold["inwT"],
        "w_s1wT": fold["s1wT"],
        "w_PT": fold["PT"],
        "w_Gv": fold["Gv"],
        "w_B8": fold["B8"],
        "w_ones64": fold["ones64"],
        "w_outwT": fold["outwT"],
        "w_freqs_dup": fold["freqs_dup"],
        "w_phase": fold["phase"],
        "w_s1b": fold["s1b"],
        "w_final_pcum": fold["final_pcum"],
        "w_epsv": np.full((G, 1), EPS, dtype=np.float32),
    }
    for i, bl in enumerate(fold["blocks"]):
        m[f"w_PC1T_{i}"] = bl["PC1T"]
        m[f"w_PMT_{i}"] = bl["PMT"]
        m[f"w_C2T_{i}"] = bl["C2T"]
        m[f"w_pd_{i}"] = bl["pd"]
        m[f"w_pcum_{i}"] = fold["pcum"][i]["pcum"]
    for name in ["up3", "up2", "up1"]:
        m[f"w_UaT_{name}"] = fold["ups"][name]["UaT"]
        m[f"w_UbT_{name}"] = fold["ups"][name]["UbT"]
    return {k: np.ascontiguousarray(v, dtype=np.float32) for k, v in m.items()}


_PROGRAM_CACHE = {}


def run_device(inp_T, sigma, params, n_tiles=NT, n_cores=N_CORES, trace=False):
    """inp_T: [66, B] fp32, sigma: [B] fp32. Returns [ACT_D, B] fp32 (no outb)."""
    key = n_tiles
    if key not in _PROGRAM_CACHE:
        _PROGRAM_CACHE[key] = _build_program(n_tiles)
    nc = _PROGRAM_CACHE[key]
    fold = _fold_params(params)
    wmap = _flat_weight_maps(fold)
    fc = n_tiles * FT
    in_maps = []
    for c in range(n_cores):
        sl = slice(c * fc, (c + 1) * fc)
        im = dict(wmap)
        im["inp"] = np.ascontiguousarray(inp_T[:, sl])
        im["sig"] = np.ascontiguousarray(sigma[None, sl])
        in_maps.append(im)
    res = run_bass_kernel_spmd(
        nc, in_maps, list(range(n_cores)), trace=trace
    )
    out = np.concatenate([res.results[c]["out"] for c in range(n_cores)], axis=1)
    return out, res, fold


def kernel(obs, noisy_action, sigma, params):
    inp_T = np.ascontiguousarray(
        np.concatenate([obs, noisy_action], axis=1).T.astype(np.float32)
    )
    sigma = np.asarray(sigma, dtype=np.float32)
    out_T, _, fold = run_device(inp_T, sigma, params)
    out = out_T.T + fold["outb"][None, :]
    return np.ascontiguousarray(out.astype(np.float32))
